# revision 2
# baseline (speedup 1.0000x reference)
"""Causal multi-head attention (B=4, T=2048, C=1024, H=16, HD=64) on 8 trn2 cores.

Sharding: core i -> batch b = i//2, head-half hh = i%2 (8 heads = 512 dims).
Each core computes q/k/v projections for its 512 head-dims, causal attention
for its 8 heads, and its partial of the output projection (Wo column block).
Host sums the two head-half partials per batch.

Per-core dataflow (PE-warmth-oriented pipeline):
  - xT (x[b].T, [1024, 2048]) resident in SBUF (f32r).
  - v for ALL 8 heads computed up-front in [t, d] layout (stationary = xT
    chunk, moving = WvT chunk), stored bf16 interleaved with a 64-wide ones
    block per head ([v_h | ones]) so each PV matmul also emits the softmax
    denominator broadcast over 64 partitions.
  - per head-pair p (2 heads): qT/kT [128, 2048] f32r via (WxT-slice).T @ xT.
  - attention in S-transposed layout per (jq, key-tile-pair): S psum
    [128, 1024] f32r matmuls column-trimmed to the causal range, exp on ACT
    (scale=1/8 fused) to bf16, diagonal 128x128 blocks masked by a
    triangular constant on GpSimd, PV accumulation in bf16.
  - normalize = DVE reciprocal + multiply writing attS (SBUF-resident, bf16)
    directly; NO DRAM spill.
  - Wo phase: y[tq-tile, c-half] = sum_p attS_p-block.T @ wo_p (bf16).
  - Software pipeline: qk-projection chains of pair p+1 are interleaved into
    attention(p) as PE filler (attention alone is ACT-bound); Wo chains fill
    attention of the last pair. Keeps TensorE dense so the HAM clock gate
    stays at 2.4 GHz.
"""

import os
import sys
from contextlib import ExitStack

import numpy as np
import ml_dtypes

try:
    from concourse import bass, tile, mybir
except ImportError:  # pragma: no cover
    sys.path.insert(0, "/opt/trn_rl_repo")
    from concourse import bass, tile, mybir

from concourse.bass2jax import _bass_exec_p, install_neuronx_cc_hook

F32 = mybir.dt.float32
F32R = mybir.dt.float32r
BF16 = mybir.dt.bfloat16
AF = mybir.ActivationFunctionType
ALU = mybir.AluOpType

B, T, C = 4, 2048, 1024
H, HD = 16, 64
NCORES = 8
HH = 512          # head-dims per core (8 heads)
NPAIR = 4         # head-pairs per core (128 dims each)
NCC = C // 128    # 8 contraction chunks for projections
NTT = T // 128    # 16 t-tiles
NTQ = T // 512    # 4 query chunks

MASK_ENGINE = "gpsimd"   # engine for the diagonal triangular mask multiply

_PROGRAM = None
last_run_info = {}


def _build_program():
    nc = bass.Bass("TRN2", target_bir_lowering=False, debug=False)

    xT_d = nc.declare_dram_parameter("xT", [C, T], F32R, isOutput=False)
    wq_d = nc.declare_dram_parameter("wq", [C, HH], F32R, isOutput=False)
    wk_d = nc.declare_dram_parameter("wk", [C, HH], F32R, isOutput=False)
    wv_d = nc.declare_dram_parameter("wv", [C, HH], F32R, isOutput=False)
    wo_d = nc.declare_dram_parameter("wo", [HH, C], BF16, isOutput=False)
    y_d = nc.declare_dram_parameter("y", [T, C], F32, isOutput=True)

    # tri[tk, tq] = 1 where tq >= tk (keep), 0 above-diagonal -> causal mask
    # for the single 128x128 block each crossing key-tile contributes.
    tri_np = np.triu(np.ones((128, 128), dtype=np.float32)).astype(
        ml_dtypes.bfloat16
    )
    tri_c = nc.inline_tensor(tri_np, "tric")

    xT = xT_d.ap()
    wq = wq_d.ap()
    wk = wk_d.ap()
    wv = wv_d.ap()
    wo = wo_d.ap()
    y = y_d.ap()

    with tile.TileContext(nc) as tc, ExitStack() as ctx:
        cst = ctx.enter_context(tc.tile_pool(name="cst", bufs=1))
        wp = ctx.enter_context(tc.tile_pool(name="wp", bufs=2))
        qkp = ctx.enter_context(tc.tile_pool(name="qkp", bufs=2))
        esp = ctx.enter_context(tc.tile_pool(name="esp", bufs=2))
        stp = ctx.enter_context(tc.tile_pool(name="stp", bufs=2))
        ybp = ctx.enter_context(tc.tile_pool(name="ybp", bufs=2))
        pp = ctx.enter_context(tc.tile_pool(name="pp", bufs=2, space="PSUM"))
        psp = ctx.enter_context(tc.tile_pool(name="psp", bufs=2, space="PSUM"))
        pap = ctx.enter_context(tc.tile_pool(name="pap", bufs=1, space="PSUM"))

        tri = cst.tile([128, 128], BF16, tag="tri")
        nc.sync.dma_start(tri[:], tri_c.ap()[:])

        xts = []
        for cc in range(NCC):
            xt = cst.tile([128, T], F32R, tag=f"x{cc}")
            nc.sync.dma_start(xt[:], xT[cc * 128:(cc + 1) * 128, :])
            xts.append(xt)

        wvs = []
        for cc in range(NCC):
            wt = cst.tile([128, HH], F32R, tag=f"wv{cc}")
            nc.sync.dma_start(wt[:], wv[cc * 128:(cc + 1) * 128, :])
            wvs.append(wt)

        wos = cst.tile([128, NPAIR, C], BF16, tag="wos")
        for p in range(NPAIR):
            nc.sync.dma_start(wos[:, p, :], wo[p * 128:(p + 1) * 128, :])

        # v_sb[:, tile, head, 0:64] = v (bf16), [..., 64:128] = ones so the
        # PV matmul's output rows 64:128 hold the softmax denominator.
        v_sb = cst.tile([128, NTT, 8, 128], BF16, tag="vsb")
        if MASK_ENGINE == "gpsimd":
            nc.gpsimd.memset(v_sb[:, :, :, 64:128], 1.0)
        else:
            nc.vector.memset(v_sb[:, :, :, 64:128], 1.0)

        attS = cst.tile([128, NPAIR, T], BF16, tag="attS")

        # --- v projection for all heads: v[tk, hd 512] per t-tile
        for tt in range(NTT):
            acc = pp.tile([128, HH], F32, tag="pp")
            for cc in range(NCC):
                nc.tensor.matmul(
                    acc[:],
                    xts[cc][:, tt * 128:(tt + 1) * 128],
                    wvs[cc][:],
                    start=(cc == 0),
                    stop=(cc == NCC - 1),
                )
            nc.vector.tensor_copy(
                v_sb[:, tt, :, 0:64],
                acc[:].rearrange("p (h d) -> p h d", h=8),
            )

        def emit_wdma(p):
            wqs = wp.tile([128, C], F32R, tag="wqs")
            wks = wp.tile([128, C], F32R, tag="wks")
            for w_sb, w_dr in ((wqs, wq), (wks, wk)):
                for cc in range(NCC):
                    nc.sync.dma_start(
                        w_sb[:, cc * 128:(cc + 1) * 128],
                        w_dr[cc * 128:(cc + 1) * 128, p * 128:(p + 1) * 128],
                    )
            return wqs, wks

        def make_proj_chains(wqs, wks):
            qT = qkp.tile([128, T], F32R, tag="qT")
            kT = qkp.tile([128, T], F32R, tag="kT")
            chains = []
            for w_sb, dst in ((wqs, qT), (wks, kT)):
                for t4 in range(NTQ):
                    def chain(w_sb=w_sb, dst=dst, t4=t4):
                        acc = pp.tile([128, 512], F32, tag="pp")
                        for cc in range(NCC):
                            nc.tensor.matmul(
                                acc[:],
                                w_sb[:, cc * 128:(cc + 1) * 128],
                                xts[cc][:, t4 * 512:(t4 + 1) * 512],
                                start=(cc == 0),
                                stop=(cc == NCC - 1),
                            )
                        nc.vector.tensor_copy(
                            dst[:, t4 * 512:(t4 + 1) * 512], acc[:]
                        )
                    chains.append(chain)
            return qT, kT, chains

        def make_wo_chain(tt, ch):
            def chain(tt=tt, ch=ch):
                yacc = pp.tile([128, 512], F32, tag="pp")
                for p in range(NPAIR):
                    nc.tensor.matmul(
                        yacc[:],
                        attS[:, p, tt * 128:(tt + 1) * 128],
                        wos[:, p, ch * 512:(ch + 1) * 512],
                        start=(p == 0),
                        stop=(p == NPAIR - 1),
                    )
                yb = ybp.tile([128, 512], F32, tag="yb")
                nc.vector.tensor_copy(yb[:], yacc[:])
                nc.sync.dma_start(
                    y[tt * 128:(tt + 1) * 128, ch * 512:(ch + 1) * 512], yb[:]
                )
            return chain

        # pair-0 projections up front (dense PE block, no filler needed yet)
        wqs0, wks0 = emit_wdma(0)
        cur_qT, cur_kT, chains0 = make_proj_chains(wqs0, wks0)
        for f in chains0:
            f()

        for p in range(NPAIR):
            if p < NPAIR - 1:
                # next pair's weights + proj chains become the PE filler for
                # this pair's (ACT-bound) attention phase.
                wqs_n, wks_n = emit_wdma(p + 1)
                qT_n, kT_n, pending = make_proj_chains(wqs_n, wks_n)
                pending = list(pending)
                per_group = 1  # emit 1 chain every other m-group (8 vs 20)
                stride = 2
            else:
                pending = []   # Wo chains appended as attS columns complete
                per_group = 2
                stride = 1

            group_i = 0
            for jq in range(NTQ):
                paA = pap.tile([128, 512], F32, tag="paA")
                paB = pap.tile([128, 512], F32, tag="paB")
                nm = 2 * jq + 2
                for m in range(nm):
                    g0 = 2 * m
                    lo = [max(0, (g0 + j - 4 * jq) * 128) for j in (0, 1)]
                    sA = psp.tile([128, 1024], F32, tag="s")
                    sB = psp.tile([128, 1024], F32, tag="s")
                    for r0, s_ in ((0, sA), (64, sB)):
                        for j in (0, 1):
                            g = g0 + j
                            nc.tensor.matmul(
                                s_[:, j * 512 + lo[j]:(j + 1) * 512],
                                cur_kT[r0:r0 + 64, g * 128:(g + 1) * 128],
                                cur_qT[r0:r0 + 64,
                                       jq * 512 + lo[j]:(jq + 1) * 512],
                                start=True,
                                stop=True,
                            )
                    eA = esp.tile([128, 1024], BF16, tag="eA")
                    eB = esp.tile([128, 1024], BF16, tag="eB")
                    nc.scalar.activation(
                        eA[:, lo[0]:1024], sA[:, lo[0]:1024], AF.Exp,
                        scale=0.125,
                    )
                    nc.scalar.activation(
                        eB[:, lo[0]:1024], sB[:, lo[0]:1024], AF.Exp,
                        scale=0.125,
                    )
                    for j in (0, 1):
                        d = g0 + j - 4 * jq
                        if d >= 0:
                            c0 = j * 512 + d * 128
                            for e_ in (eA, eB):
                                eng = (
                                    nc.gpsimd
                                    if MASK_ENGINE == "gpsimd"
                                    else nc.vector
                                )
                                eng.tensor_tensor(
                                    e_[:, c0:c0 + 128],
                                    e_[:, c0:c0 + 128],
                                    tri[:],
                                    ALU.mult,
                                )
                    for hh_i, (e_, pa_) in enumerate(((eA, paA), (eB, paB))):
                        gh = 2 * p + hh_i
                        for j in (0, 1):
                            g = g0 + j
                            nc.tensor.matmul(
                                pa_[:, lo[j]:512],
                                v_sb[:, g, gh, :],
                                e_[:, j * 512 + lo[j]:(j + 1) * 512],
                                start=(m == 0 and j == 0),
                                stop=(m == nm - 1 and j == 1),
                            )
                    group_i += 1
                    if group_i % stride == 0:
                        for _ in range(per_group):
                            if pending:
                                pending.pop(0)()
                # normalize: rows 64:128 hold the broadcast denominator
                for hh_i, pa_ in ((0, paA), (1, paB)):
                    rc = stp.tile([64, 512], F32, tag="rc")
                    nc.vector.reciprocal(rc[:], pa_[64:128, :])
                    nc.vector.tensor_tensor(
                        attS[hh_i * 64:(hh_i + 1) * 64, p,
                             jq * 512:(jq + 1) * 512],
                        pa_[0:64, :],
                        rc[:],
                        ALU.mult,
                    )
                if p == NPAIR - 1:
                    for tt in range(4 * jq, 4 * jq + 4):
                        for ch in range(2):
                            pending.append(make_wo_chain(tt, ch))
            # all of next pair's projections must exist before its attention
            while pending:
                pending.pop(0)()
            if p < NPAIR - 1:
                cur_qT, cur_kT = qT_n, kT_n

    _split_matmul_waits(nc)
    return nc


def _split_matmul_waits(nc):
    """walrus's fp32r fused-LDW matmul lowering can't carry multiple sync
    waits (S3_LW setupSyncWait assert). Move every matmul's waits onto a
    preceding same-engine NoOp, which lowers with full sync support."""
    f = nc.m.functions[0]
    k = 0
    for bb in f.blocks:
        insts = bb.instructions
        out = []
        for i in insts:
            waits = list(i.sync_info.on_wait) if i.sync_info is not None else []
            keep = 0 if type(i).__name__ == "InstMatmult" else 1
            if len(waits) > keep:
                moved, kept = waits[: len(waits) - keep], waits[len(waits) - keep:]
                for w in moved:
                    n = mybir.InstNoOp(name=f"I-mmwait{k}")
                    k += 1
                    n.engine = i.engine
                    n.sync_info = mybir.SyncInfo(on_wait=[w], on_update=[])
                    nc.register_instruction(n)
                    out.append(n)
                i.sync_info = mybir.SyncInfo(
                    on_wait=kept, on_update=list(i.sync_info.on_update)
                )
            out.append(i)
        if k:
            bb.instructions = out


def _get_program():
    global _PROGRAM
    if _PROGRAM is None:
        _PROGRAM = _build_program()
    return _PROGRAM


_RUNNER = None


def _get_runner():
    """Compile the SPMD program into a cached sharded jit callable.

    Mirrors bass2jax.run_bass_via_pjrt's multi-core branch, but without
    output donation (y is fully written by the kernel) so the callable can
    be re-invoked for timing without re-staging zero buffers.
    """
    global _RUNNER
    if _RUNNER is not None:
        return _RUNNER
    import jax
    from jax.experimental.shard_map import shard_map
    from jax.sharding import Mesh, PartitionSpec

    nc = _get_program()
    install_neuronx_cc_hook()

    partition_name = (
        nc.partition_id_tensor.name if nc.partition_id_tensor else None
    )
    in_names, out_names, out_avals = [], [], []
    for alloc in nc.m.functions[0].allocations:
        if not isinstance(alloc, mybir.MemoryLocationSet):
            continue
        name = alloc.memorylocations[0].name
        if alloc.kind == "ExternalInput":
            if name != partition_name:
                in_names.append(name)
        elif alloc.kind == "ExternalOutput":
            out_names.append(name)
            out_avals.append(
                jax.core.ShapedArray(tuple(alloc.tensor_shape), mybir.dt.np(alloc.dtype))
            )
    n_params = len(in_names)
    zero_outs = [np.zeros(a.shape, a.dtype) for a in out_avals]
    all_in_names = list(in_names) + list(out_names)
    if partition_name is not None:
        all_in_names.append(partition_name)
    all_in_names = tuple(all_in_names)

    def _body(*args):
        operands = list(args)
        if partition_name is not None:
            from concourse.bass2jax import partition_id_tensor

            operands.append(partition_id_tensor())
        outs = _bass_exec_p.bind(
            *operands,
            out_avals=tuple(out_avals),
            in_names=all_in_names,
            out_names=tuple(out_names),
            lowering_input_output_aliases=(),
            sim_require_finite=True,
            sim_require_nnan=True,
            nc=nc,
        )
        return tuple(outs)

    devices = jax.devices()[:NCORES]
    assert len(devices) == NCORES, devices
    mesh = Mesh(np.asarray(devices), ("core",))
    n_all = n_params + len(out_names)
    sharded = jax.jit(
        shard_map(
            _body,
            mesh=mesh,
            in_specs=(PartitionSpec("core"),) * n_all,
            out_specs=(PartitionSpec("core"),) * len(out_names),
            check_rep=False,
        ),
        keep_unused=True,
    )
    _RUNNER = dict(
        sharded=sharded,
        in_names=in_names,
        out_names=out_names,
        out_avals=out_avals,
        zero_outs=zero_outs,
        mesh=mesh,
    )
    return _RUNNER


def _run(in_maps):
    r = _get_runner()
    concat_in = [
        np.concatenate([np.asarray(m[name]) for m in in_maps], axis=0)
        for name in r["in_names"]
    ]
    concat_zeros = [
        np.zeros((NCORES * z.shape[0], *z.shape[1:]), z.dtype) for z in r["zero_outs"]
    ]
    out_arrs = r["sharded"](*concat_in, *concat_zeros)
    return [
        {
            name: np.asarray(out_arrs[i]).reshape(NCORES, *r["out_avals"][i].shape)[c]
            for i, name in enumerate(r["out_names"])
        }
        for c in range(NCORES)
    ]


def timed_run(in_maps, iters=10):
    """Execute with inputs pre-staged on device; return per-iteration seconds."""
    import time
    import jax

    r = _get_runner()
    concat_in = [
        np.concatenate([np.asarray(m[name]) for m in in_maps], axis=0)
        for name in r["in_names"]
    ]
    concat_zeros = [
        np.zeros((NCORES * z.shape[0], *z.shape[1:]), z.dtype) for z in r["zero_outs"]
    ]
    from jax.sharding import NamedSharding, PartitionSpec

    sh = NamedSharding(r["mesh"], PartitionSpec("core"))
    args = [jax.device_put(a, sh) for a in concat_in + concat_zeros]
    out = r["sharded"](*args)  # warmup + compile
    jax.block_until_ready(out)
    times = []
    for _ in range(iters):
        t0 = time.perf_counter()
        out = r["sharded"](*args)
        jax.block_until_ready(out)
        times.append(time.perf_counter() - t0)
    return times


def round_f32r(a):
    """Round fp32 to the float32r grid: 11 explicit mantissa bits (RNE),
    low 12 bits zeroed — matching walrus's cast_fp32_to_fp32r."""
    a = np.ascontiguousarray(a, dtype=np.float32)
    u = a.view(np.uint32)
    lsb = (u >> np.uint32(12)) & np.uint32(1)
    u2 = (u + np.uint32(0x7FF) + lsb) & np.uint32(0xFFFFF000)
    return u2.view(np.float32)


def make_in_maps(x, Wq, Wk, Wv, Wo):
    x = np.asarray(x, dtype=np.float32)
    Wq = np.asarray(Wq, dtype=np.float32)
    Wk = np.asarray(Wk, dtype=np.float32)
    Wv = np.asarray(Wv, dtype=np.float32)
    Wo = np.asarray(Wo, dtype=np.float32)
    xTs = [round_f32r(np.ascontiguousarray(x[b].T)) for b in range(B)]
    in_maps = []
    for core in range(NCORES):
        b, hh = core // 2, core % 2
        sl = slice(hh * HH, (hh + 1) * HH)
        in_maps.append({
            "xT": xTs[b],
            "wq": round_f32r(np.ascontiguousarray(Wq[sl, :].T)),
            "wk": round_f32r(np.ascontiguousarray(Wk[sl, :].T)),
            "wv": round_f32r(np.ascontiguousarray(Wv[sl, :].T)),
            "wo": np.ascontiguousarray(Wo[:, sl].T).astype(ml_dtypes.bfloat16),
        })
    return in_maps


def kernel(x, Wq, Wk, Wv, Wo):
    in_maps = make_in_maps(x, Wq, Wk, Wv, Wo)
    results = _run(in_maps)
    out = np.empty((B, T, C), dtype=np.float32)
    for b in range(B):
        out[b] = results[2 * b]["y"] + results[2 * b + 1]["y"]
    return out


# revision 10
# speedup vs baseline: 1.3682x; 1.3682x over previous
"""Causal multi-head attention (B=4, T=2048, C=1024, H=16, HD=64) on 8 trn2 cores.

Sharding: core i -> batch b = i//2, head-half hh = i%2 (8 heads = 512 dims).
Each core computes q/k/v projections for its 512 head-dims, causal attention
for its 8 heads, and its partial of the output projection (Wo column block).
Host sums the two head-half partials per batch.

Per-core dataflow (PE-warmth-oriented pipeline):
  - xT (x[b].T, [1024, 2048]) resident in SBUF (f32r).
  - v for ALL 8 heads computed up-front in [t, d] layout (stationary = xT
    chunk, moving = WvT chunk), stored bf16 interleaved with a 64-wide ones
    block per head ([v_h | ones]) so each PV matmul also emits the softmax
    denominator broadcast over 64 partitions.
  - per head-pair p (2 heads): qT/kT [128, 2048] f32r via (WxT-slice).T @ xT.
  - attention in S-transposed layout per (jq, key-tile-pair): S psum
    [128, 1024] f32r matmuls column-trimmed to the causal range, exp on ACT
    (scale=1/8 fused) to bf16, diagonal 128x128 blocks masked by a
    triangular constant on GpSimd, PV accumulation in bf16.
  - normalize = DVE reciprocal + multiply writing attS (SBUF-resident, bf16)
    directly; NO DRAM spill.
  - Wo phase: y[tq-tile, c-half] = sum_p attS_p-block.T @ wo_p (bf16).
  - Software pipeline: qk-projection chains of pair p+1 are interleaved into
    attention(p) as PE filler (attention alone is ACT-bound); Wo chains fill
    attention of the last pair. Keeps TensorE dense so the HAM clock gate
    stays at 2.4 GHz.
"""

import os
import sys
from contextlib import ExitStack

import numpy as np

try:
    from concourse import bass, tile, mybir
except ImportError:  # pragma: no cover
    sys.path.insert(0, "/opt/trn_rl_repo")
    from concourse import bass, tile, mybir

from concourse.bass2jax import _bass_exec_p, install_neuronx_cc_hook

F32 = mybir.dt.float32
F16 = mybir.dt.float16
AF = mybir.ActivationFunctionType
ALU = mybir.AluOpType

B, T, C = 4, 2048, 1024
H, HD = 16, 64
NCORES = 8
HH = 512          # head-dims per core (8 heads)
NPAIR = 4         # head-pairs per core (128 dims each)
NCC = C // 128    # 8 contraction chunks for projections
NTT = T // 128    # 16 t-tiles
NTQ = T // 512    # 4 query chunks

MASK_ENGINE = "gpsimd"   # engine for the diagonal triangular mask multiply

_PROGRAM = None
last_run_info = {}


def _build_program():
    nc = bass.Bass("TRN2", target_bir_lowering=False, debug=False)

    xT_d = nc.declare_dram_parameter("xT", [C, T], F16, isOutput=False)
    wq_d = nc.declare_dram_parameter("wq", [C, HH], F16, isOutput=False)
    wk_d = nc.declare_dram_parameter("wk", [C, HH], F16, isOutput=False)
    wv_d = nc.declare_dram_parameter("wv", [C, HH], F16, isOutput=False)
    wo_d = nc.declare_dram_parameter("wo", [HH, C], F16, isOutput=False)
    y_d = nc.declare_dram_parameter("y", [T, C], F32, isOutput=True)

    # tri[tk, tq] = 1 where tq >= tk (keep), 0 above-diagonal -> causal mask
    # for the single 128x128 block each crossing key-tile contributes.
    tri_np = np.triu(np.ones((128, 128), dtype=np.float16))
    tri_c = nc.inline_tensor(tri_np, "tric")

    xT = xT_d.ap()
    wq = wq_d.ap()
    wk = wk_d.ap()
    wv = wv_d.ap()
    wo = wo_d.ap()
    y = y_d.ap()

    with tile.TileContext(nc) as tc, ExitStack() as ctx:
        cst = ctx.enter_context(tc.tile_pool(name="cst", bufs=1))
        wp = ctx.enter_context(tc.tile_pool(name="wp", bufs=2))
        qkp = ctx.enter_context(tc.tile_pool(name="qkp", bufs=2))
        esp = ctx.enter_context(tc.tile_pool(name="esp", bufs=2))
        stp = ctx.enter_context(tc.tile_pool(name="stp", bufs=2))
        ybp = ctx.enter_context(tc.tile_pool(name="ybp", bufs=2))
        pp = ctx.enter_context(tc.tile_pool(name="pp", bufs=2, space="PSUM"))
        psp = ctx.enter_context(tc.tile_pool(name="psp", bufs=2, space="PSUM"))
        pap = ctx.enter_context(tc.tile_pool(name="pap", bufs=1, space="PSUM"))

        tri = cst.tile([128, 128], F16, tag="tri")
        nc.sync.dma_start(tri[:], tri_c.ap()[:])

        xts = []
        for cc in range(NCC):
            xt = cst.tile([128, T], F16, tag=f"x{cc}")
            nc.sync.dma_start(xt[:], xT[cc * 128:(cc + 1) * 128, :])
            xts.append(xt)

        wvs = []
        for cc in range(NCC):
            wt = cst.tile([128, HH], F16, tag=f"wv{cc}")
            nc.sync.dma_start(wt[:], wv[cc * 128:(cc + 1) * 128, :])
            wvs.append(wt)

        wos = cst.tile([128, NPAIR, C], F16, tag="wos")
        for p in range(NPAIR):
            nc.sync.dma_start(wos[:, p, :], wo[p * 128:(p + 1) * 128, :])

        # v_sb[:, tile, pair, parity, :]: even heads hold [v | ones], odd
        # heads [ones | v], so each PV matmul also emits the softmax
        # denominator — on rows 64:128 for even heads, rows 0:64 for odd.
        # The flipped parity lets normalize gather att rows of both heads
        # into one [128, 512] tile with same-partition copies only.
        v_sb = cst.tile([128, NTT, 4, 2, 128], F16, tag="vsb")
        nc.gpsimd.memset(v_sb[:, :, :, 0, 64:128], 1.0)
        nc.gpsimd.memset(v_sb[:, :, :, 1, 0:64], 1.0)

        attS = cst.tile([128, NPAIR, T], F16, tag="attS")

        # --- v projection for all heads: v[tk, hd 512] per t-tile
        for tt in range(NTT):
            acc = pp.tile([128, HH], F32, tag="pp")
            for cc in range(NCC):
                nc.tensor.matmul(
                    acc[:],
                    xts[cc][:, tt * 128:(tt + 1) * 128],
                    wvs[cc][:],
                    start=(cc == 0),
                    stop=(cc == NCC - 1),
                )
            av = acc[:].rearrange("p (h4 two d) -> p h4 two d", h4=4, two=2)
            nc.vector.tensor_copy(v_sb[:, tt, :, 0, 0:64], av[:, :, 0, :])
            nc.vector.tensor_copy(v_sb[:, tt, :, 1, 64:128], av[:, :, 1, :])

        def emit_wdma(p):
            wqs = wp.tile([128, C], F16, tag="wqs")
            wks = wp.tile([128, C], F16, tag="wks")
            for w_sb, w_dr in ((wqs, wq), (wks, wk)):
                for cc in range(NCC):
                    nc.sync.dma_start(
                        w_sb[:, cc * 128:(cc + 1) * 128],
                        w_dr[cc * 128:(cc + 1) * 128, p * 128:(p + 1) * 128],
                    )
            return wqs, wks

        def make_proj_chains(wqs, wks):
            qT = qkp.tile([128, T], F16, tag="qT")
            kT = qkp.tile([128, T], F16, tag="kT")
            chains = []
            for w_sb, dst in ((wqs, qT), (wks, kT)):
                for t4 in range(NTQ):
                    def chain(w_sb=w_sb, dst=dst, t4=t4):
                        acc = pp.tile([128, 512], F32, tag="pp")
                        for cc in range(NCC):
                            nc.tensor.matmul(
                                acc[:],
                                w_sb[:, cc * 128:(cc + 1) * 128],
                                xts[cc][:, t4 * 512:(t4 + 1) * 512],
                                start=(cc == 0),
                                stop=(cc == NCC - 1),
                            )
                        nc.vector.tensor_copy(
                            dst[:, t4 * 512:(t4 + 1) * 512], acc[:]
                        )
                    chains.append(chain)
            return qT, kT, chains

        def make_wo_chain(tt, ch):
            def chain(tt=tt, ch=ch):
                yacc = pp.tile([128, 512], F32, tag="pp")
                for p in range(NPAIR):
                    nc.tensor.matmul(
                        yacc[:],
                        attS[:, p, tt * 128:(tt + 1) * 128],
                        wos[:, p, ch * 512:(ch + 1) * 512],
                        start=(p == 0),
                        stop=(p == NPAIR - 1),
                    )
                yb = ybp.tile([128, 512], F32, tag="yb")
                nc.vector.tensor_copy(yb[:], yacc[:])
                nc.sync.dma_start(
                    y[tt * 128:(tt + 1) * 128, ch * 512:(ch + 1) * 512], yb[:]
                )
            return chain

        # pair-0 projections up front (dense PE block, no filler needed yet)
        wqs0, wks0 = emit_wdma(0)
        cur_qT, cur_kT, chains0 = make_proj_chains(wqs0, wks0)
        for f in chains0:
            f()

        for p in range(NPAIR):
            if p < NPAIR - 1:
                # next pair's weights + proj chains become the PE filler for
                # this pair's (ACT-bound) attention phase.
                wqs_n, wks_n = emit_wdma(p + 1)
                qT_n, kT_n, pending = make_proj_chains(wqs_n, wks_n)
                pending = list(pending)
                per_group = 1  # emit 1 chain every other m-group (8 vs 20)
                stride = 2
            else:
                pending = []   # Wo chains appended as attS columns complete
                per_group = 2
                stride = 1

            group_i = 0
            for jq in range(NTQ):
                paA = pap.tile([128, 512], F32, tag="paA")
                paB = pap.tile([128, 512], F32, tag="paB")
                nm = 2 * jq + 2
                for m in range(nm):
                    g0 = 2 * m
                    lo = [max(0, (g0 + j - 4 * jq) * 128) for j in (0, 1)]
                    sA = psp.tile([128, 1024], F32, tag="s")
                    sB = psp.tile([128, 1024], F32, tag="s")
                    for r0, s_ in ((0, sA), (64, sB)):
                        for j in (0, 1):
                            g = g0 + j
                            nc.tensor.matmul(
                                s_[:, j * 512 + lo[j]:(j + 1) * 512],
                                cur_kT[r0:r0 + 64, g * 128:(g + 1) * 128],
                                cur_qT[r0:r0 + 64,
                                       jq * 512 + lo[j]:(jq + 1) * 512],
                                start=True,
                                stop=True,
                            )
                    eA = esp.tile([128, 1024], F16, tag="eA")
                    eB = esp.tile([128, 1024], F16, tag="eB")
                    nc.scalar.activation(
                        eA[:, lo[0]:1024], sA[:, lo[0]:1024], AF.Exp,
                        scale=0.125,
                    )
                    nc.scalar.activation(
                        eB[:, lo[0]:1024], sB[:, lo[0]:1024], AF.Exp,
                        scale=0.125,
                    )
                    for j in (0, 1):
                        d = g0 + j - 4 * jq
                        if d >= 0:
                            c0 = j * 512 + d * 128
                            for e_ in (eA, eB):
                                eng = (
                                    nc.gpsimd
                                    if MASK_ENGINE == "gpsimd"
                                    else nc.vector
                                )
                                eng.tensor_tensor(
                                    e_[:, c0:c0 + 128],
                                    e_[:, c0:c0 + 128],
                                    tri[:],
                                    ALU.mult,
                                )
                    for hh_i, (e_, pa_) in enumerate(((eA, paA), (eB, paB))):
                        for j in (0, 1):
                            g = g0 + j
                            nc.tensor.matmul(
                                pa_[:, lo[j]:512],
                                v_sb[:, g, p, hh_i, :],
                                e_[:, j * 512 + lo[j]:(j + 1) * 512],
                                start=(m == 0 and j == 0),
                                stop=(m == nm - 1 and j == 1),
                            )
                    group_i += 1
                    if group_i % stride == 0:
                        for _ in range(per_group):
                            if pending:
                                pending.pop(0)()
                # normalize: paA = [attA | denA], paB = [denB | attB].
                # Same-partition copies gather att/den into base-0 tiles
                # (freeing the pa psum banks fast), then one GpSimd divide
                # writes both heads' normalized rows into attS.
                num2 = stp.tile([128, 512], F32, tag="num")
                den2 = stp.tile([128, 512], F32, tag="den")
                lden = stp.tile([128, 512], F32, tag="lden")
                rc2 = stp.tile([128, 512], F32, tag="rc2")
                nc.vector.tensor_copy(num2[0:64, :], paA[0:64, :])
                nc.vector.tensor_copy(den2[64:128, :], paA[64:128, :])
                nc.vector.tensor_copy(den2[0:64, :], paB[0:64, :])
                nc.vector.tensor_copy(num2[64:128, :], paB[64:128, :])
                # 1/den as exp(-ln(den)) on ACT (no fast divide on any
                # engine; DVE InstReciprocal is ~8 cycles/elem and was the
                # jq-boundary serializer that kept re-throttling the PE).
                nc.scalar.activation(lden[:], den2[:], AF.Ln)
                nc.scalar.activation(rc2[:], lden[:], AF.Exp, scale=-1.0)
                # den2/rc2 rows are [denB | denA] (same-partition copies
                # can't shift partitions); DMA's crossbar swaps the halves
                # so each head multiplies by its own reciprocal.
                rc2s = stp.tile([128, 512], F32, tag="rc2s")
                nc.sync.dma_start(rc2s[0:64, :], rc2[64:128, :])
                nc.sync.dma_start(rc2s[64:128, :], rc2[0:64, :])
                nc.gpsimd.tensor_tensor(
                    attS[:, p, jq * 512:(jq + 1) * 512],
                    num2[:],
                    rc2s[:],
                    ALU.mult,
                )
                if p == NPAIR - 1:
                    for tt in range(4 * jq, 4 * jq + 4):
                        for ch in range(2):
                            pending.append(make_wo_chain(tt, ch))
            # all of next pair's projections must exist before its attention
            while pending:
                pending.pop(0)()
            if p < NPAIR - 1:
                cur_qT, cur_kT = qT_n, kT_n

    _split_matmul_waits(nc)
    return nc


def _split_matmul_waits(nc):
    """walrus's fp32r fused-LDW matmul lowering can't carry multiple sync
    waits (S3_LW setupSyncWait assert). Move every matmul's waits onto a
    preceding same-engine NoOp, which lowers with full sync support."""
    f = nc.m.functions[0]
    k = 0
    for bb in f.blocks:
        insts = bb.instructions
        out = []
        for i in insts:
            waits = list(i.sync_info.on_wait) if i.sync_info is not None else []
            keep = 0 if type(i).__name__ == "InstMatmult" else 1
            if len(waits) > keep:
                moved, kept = waits[: len(waits) - keep], waits[len(waits) - keep:]
                for w in moved:
                    n = mybir.InstNoOp(name=f"I-mmwait{k}")
                    k += 1
                    n.engine = i.engine
                    n.sync_info = mybir.SyncInfo(on_wait=[w], on_update=[])
                    nc.register_instruction(n)
                    out.append(n)
                i.sync_info = mybir.SyncInfo(
                    on_wait=kept, on_update=list(i.sync_info.on_update)
                )
            out.append(i)
        if k:
            bb.instructions = out


def _get_program():
    global _PROGRAM
    if _PROGRAM is None:
        _PROGRAM = _build_program()
    return _PROGRAM


_RUNNER = None


def _get_runner():
    """Compile the SPMD program into a cached sharded jit callable.

    Mirrors bass2jax.run_bass_via_pjrt's multi-core branch, but without
    output donation (y is fully written by the kernel) so the callable can
    be re-invoked for timing without re-staging zero buffers.
    """
    global _RUNNER
    if _RUNNER is not None:
        return _RUNNER
    import jax
    from jax.experimental.shard_map import shard_map
    from jax.sharding import Mesh, PartitionSpec

    nc = _get_program()
    install_neuronx_cc_hook()

    partition_name = (
        nc.partition_id_tensor.name if nc.partition_id_tensor else None
    )
    in_names, out_names, out_avals = [], [], []
    for alloc in nc.m.functions[0].allocations:
        if not isinstance(alloc, mybir.MemoryLocationSet):
            continue
        name = alloc.memorylocations[0].name
        if alloc.kind == "ExternalInput":
            if name != partition_name:
                in_names.append(name)
        elif alloc.kind == "ExternalOutput":
            out_names.append(name)
            out_avals.append(
                jax.core.ShapedArray(tuple(alloc.tensor_shape), mybir.dt.np(alloc.dtype))
            )
    n_params = len(in_names)
    zero_outs = [np.zeros(a.shape, a.dtype) for a in out_avals]
    all_in_names = list(in_names) + list(out_names)
    if partition_name is not None:
        all_in_names.append(partition_name)
    all_in_names = tuple(all_in_names)

    def _body(*args):
        operands = list(args)
        if partition_name is not None:
            from concourse.bass2jax import partition_id_tensor

            operands.append(partition_id_tensor())
        outs = _bass_exec_p.bind(
            *operands,
            out_avals=tuple(out_avals),
            in_names=all_in_names,
            out_names=tuple(out_names),
            lowering_input_output_aliases=(),
            sim_require_finite=True,
            sim_require_nnan=True,
            nc=nc,
        )
        return tuple(outs)

    devices = jax.devices()[:NCORES]
    assert len(devices) == NCORES, devices
    mesh = Mesh(np.asarray(devices), ("core",))
    n_all = n_params + len(out_names)
    sharded = jax.jit(
        shard_map(
            _body,
            mesh=mesh,
            in_specs=(PartitionSpec("core"),) * n_all,
            out_specs=(PartitionSpec("core"),) * len(out_names),
            check_rep=False,
        ),
        keep_unused=True,
    )
    _RUNNER = dict(
        sharded=sharded,
        in_names=in_names,
        out_names=out_names,
        out_avals=out_avals,
        zero_outs=zero_outs,
        mesh=mesh,
    )
    return _RUNNER


def _run(in_maps):
    r = _get_runner()
    concat_in = [
        np.concatenate([np.asarray(m[name]) for m in in_maps], axis=0)
        for name in r["in_names"]
    ]
    concat_zeros = [
        np.zeros((NCORES * z.shape[0], *z.shape[1:]), z.dtype) for z in r["zero_outs"]
    ]
    out_arrs = r["sharded"](*concat_in, *concat_zeros)
    return [
        {
            name: np.asarray(out_arrs[i]).reshape(NCORES, *r["out_avals"][i].shape)[c]
            for i, name in enumerate(r["out_names"])
        }
        for c in range(NCORES)
    ]


def timed_run(in_maps, iters=10):
    """Execute with inputs pre-staged on device; return per-iteration seconds."""
    import time
    import jax

    r = _get_runner()
    concat_in = [
        np.concatenate([np.asarray(m[name]) for m in in_maps], axis=0)
        for name in r["in_names"]
    ]
    concat_zeros = [
        np.zeros((NCORES * z.shape[0], *z.shape[1:]), z.dtype) for z in r["zero_outs"]
    ]
    from jax.sharding import NamedSharding, PartitionSpec

    sh = NamedSharding(r["mesh"], PartitionSpec("core"))
    args = [jax.device_put(a, sh) for a in concat_in + concat_zeros]
    out = r["sharded"](*args)  # warmup + compile
    jax.block_until_ready(out)
    times = []
    for _ in range(iters):
        t0 = time.perf_counter()
        out = r["sharded"](*args)
        jax.block_until_ready(out)
        times.append(time.perf_counter() - t0)
    return times


def round_f32r(a):
    """Round fp32 to the float32r grid: 11 explicit mantissa bits (RNE),
    low 12 bits zeroed — matching walrus's cast_fp32_to_fp32r."""
    a = np.ascontiguousarray(a, dtype=np.float32)
    u = a.view(np.uint32)
    lsb = (u >> np.uint32(12)) & np.uint32(1)
    u2 = (u + np.uint32(0x7FF) + lsb) & np.uint32(0xFFFFF000)
    return u2.view(np.float32)


def make_in_maps(x, Wq, Wk, Wv, Wo):
    x = np.asarray(x, dtype=np.float32)
    Wq = np.asarray(Wq, dtype=np.float32)
    Wk = np.asarray(Wk, dtype=np.float32)
    Wv = np.asarray(Wv, dtype=np.float32)
    Wo = np.asarray(Wo, dtype=np.float32)
    xTs = [np.ascontiguousarray(x[b].T).astype(np.float16) for b in range(B)]
    in_maps = []
    for core in range(NCORES):
        b, hh = core // 2, core % 2
        sl = slice(hh * HH, (hh + 1) * HH)
        in_maps.append({
            "xT": xTs[b],
            "wq": np.ascontiguousarray(Wq[sl, :].T).astype(np.float16),
            "wk": np.ascontiguousarray(Wk[sl, :].T).astype(np.float16),
            "wv": np.ascontiguousarray(Wv[sl, :].T).astype(np.float16),
            "wo": np.ascontiguousarray(Wo[:, sl].T).astype(np.float16),
        })
    return in_maps


def kernel(x, Wq, Wk, Wv, Wo):
    in_maps = make_in_maps(x, Wq, Wk, Wv, Wo)
    results = _run(in_maps)
    out = np.empty((B, T, C), dtype=np.float32)
    for b in range(B):
        out[b] = results[2 * b]["y"] + results[2 * b + 1]["y"]
    return out
